# revision 1
# baseline (speedup 1.0000x reference)
"""Multi-head causal attention (B=4, S=2048, D=768, H=4 heads) on 8 TRN2 cores.

Sharding: core c handles batch b = c//2 and head-pair hp = c%2 (heads 2*hp,
2*hp+1).  Each core projects x[b] through its 384-column slice of Wq/Wk/Wv,
runs causal attention for its two heads, and pushes the result through its
384-row slice of Wo.  The host sums the two partial outputs per batch and
adds bo.  This splits every matmul's FLOPs exactly 8 ways with no duplicated
work and needs no device collectives.

Dataflow is kept transposed end-to-end ([feature, seq] layouts) so the kernel
needs zero on-device transposes:
  QT/KT = W^T x^T           [384, S]   (3 chunks of 128 partitions)
  V     = x W               [S, 384]   (16 chunks of 128 partitions, with a
                                        ones column appended per head so the
                                        softmax denominator falls out of the
                                        ctx matmul as one extra output row)
  S^T   = KT'Q              [k, q]     k on partitions -> softmax sum over k
  ctx^T = V^T E             [192+1, q]
  out^T = Wo^T ctx^T        [768, S]
Causal structure: key-tile i (128 rows) x query-tile j (512 cols) blocks with
i > 4j+3 are fully masked and skipped entirely; diagonal blocks get a 0/1
mask multiply after exp.  Scores are O(1) so exp needs no max-subtraction.

The query-tile loop is software-pipelined so the (DVE-heavy) softmax
normalization of tile j-1 overlaps the (PE-heavy) score phase of tile j,
keeping TensorE dense and the HAM clock-gate warm.

Matmul operands are fp16 (PSUM accumulates fp32).
"""

import sys

for _p in ("/opt/trn_rl_repo",):
    if _p not in sys.path:
        sys.path.insert(0, _p)

import numpy as np

S = 2048            # sequence length
D = 768             # model dim
DH = 192            # head dim
DD = 2 * DH         # feature columns per core (2 heads)
P = 128             # partitions
KC = D // P         # 6 contraction chunks over D
MC = DD // P        # 3 chunks over the per-core head dims
QT = 512            # query tile (matmul free dim, one PSUM bank)
NQ = S // QT        # 4 query tiles
NK = S // P         # 16 key tiles
SCALE = 1.0 / float(np.sqrt(DH))

# Per-head slices of the [384 -> 3x128chunk] QT/KT layout, ordered so the two
# K=64 pieces of the two heads land in different PE row groups (base partition
# 0 vs 64) and can overlap in the array.
#   h=0: chunk0 rows 0:128  +  chunk1 rows 0:64
#   h=1: chunk2 rows 0:128  +  chunk1 rows 64:128
HEAD_PIECES = [
    [(0, 0, 128), (1, 0, 64)],
    [(2, 0, 128), (1, 64, 64)],
]

_CACHE = {}


def _build_nc():
    import concourse.bacc as bacc
    import concourse.tile as tile
    from concourse import mybir

    F16 = mybir.dt.float16
    F32 = mybir.dt.float32
    EXP = mybir.ActivationFunctionType.Exp
    IDENT = mybir.ActivationFunctionType.Identity

    nc = bacc.Bacc(None, target_bir_lowering=False)

    xt = nc.dram_tensor("xt", [P, KC, S], F16, kind="ExternalInput")
    wq = nc.dram_tensor("wq", [P, KC, DD], F16, kind="ExternalInput")
    wk = nc.dram_tensor("wk", [P, KC, DD], F16, kind="ExternalInput")
    wv = nc.dram_tensor("wv", [P, KC, DD], F16, kind="ExternalInput")
    wo = nc.dram_tensor("wo", [P, 4, D], F16, kind="ExternalInput")
    bqk = nc.dram_tensor("bqk", [P, 6], F32, kind="ExternalInput")
    bvr = nc.dram_tensor("bvr", [1, DD], F32, kind="ExternalInput")
    msk = nc.dram_tensor("msk", [P, 4, QT], F16, kind="ExternalInput")
    out_t = nc.dram_tensor("out_t", [P, KC, S], F32, kind="ExternalOutput")

    with tile.TileContext(nc) as tc:
        with (
            tc.tile_pool(name="persist", bufs=1) as pp,
            tc.tile_pool(name="epool", bufs=56) as ep,
            tc.tile_pool(name="ctxp", bufs=3) as cp,
            tc.tile_pool(name="workp", bufs=2) as wp,
            tc.tile_pool(name="outp", bufs=3) as op_,
            tc.tile_pool(name="psA", bufs=3, space="PSUM") as psA,
            tc.tile_pool(name="psC", bufs=2, space="PSUM") as psC,
        ):
            # ---- loads, split across both HWDGE rings (sync + scalar).
            # The critical path is wq + the first x quarter (the first QT
            # matmuls); those go first, with the x quarter split across both
            # rings so it finishes soonest.
            x_sb = pp.tile([P, KC, S], F16)
            wq_sb = pp.tile([P, KC, DD], F16)
            wk_sb = pp.tile([P, KC, DD], F16)
            nc.sync.dma_start(out=x_sb[:, 0:3, 0:QT], in_=xt[:, 0:3, 0:QT])
            nc.scalar.dma_start(out=x_sb[:, 3:6, 0:QT], in_=xt[:, 3:6, 0:QT])
            nc.sync.dma_start(out=wq_sb, in_=wq[:, :, :])
            nc.scalar.dma_start(out=wk_sb, in_=wk[:, :, :])
            nc.sync.dma_start(out=x_sb[:, :, QT : 2 * QT], in_=xt[:, :, QT : 2 * QT])
            bqk_sb = pp.tile([P, 6], F32)
            nc.scalar.dma_start(out=bqk_sb, in_=bqk[:, :])
            bvr_sb = pp.tile([1, DD], F32)
            nc.scalar.dma_start(out=bvr_sb, in_=bvr[:, :])
            nc.sync.dma_start(
                out=x_sb[:, :, 2 * QT : 3 * QT], in_=xt[:, :, 2 * QT : 3 * QT]
            )
            wv_sb = pp.tile([P, KC, DD], F16)
            nc.scalar.dma_start(out=wv_sb, in_=wv[:, :, :])
            nc.sync.dma_start(out=x_sb[:, :, 3 * QT : S], in_=xt[:, :, 3 * QT : S])
            wo_sb = pp.tile([P, 4, D], F16)
            nc.scalar.dma_start(out=wo_sb, in_=wo[:, :, :])
            msk_sb = pp.tile([P, 4, QT], F16)
            nc.scalar.dma_start(out=msk_sb, in_=msk[:, :, :])

            ones_sb = pp.tile([1, P], F16)
            nc.vector.memset(ones_sb, 1.0)

            # Dummy matmuls on the just-memset ones tile: keeps TensorE busy
            # during the initial loads so the HAM clock-gate is already warm
            # (2.4 GHz) when the first real matmul issues.
            ps_warm = psA.tile([P, QT], F32, tag="mm", name="ps_warm")
            for w in range(64):
                nc.tensor.matmul(
                    ps_warm[:, 0:P], lhsT=ones_sb, rhs=ones_sb,
                    start=True, stop=True, skip_group_check=(w > 0),
                )

            # ---- Q^T and K^T projections: [384(3x128), 2048] fp16 ----
            # (bias-add + fp16 cast on ScalarE to keep DVE free)
            qt_sb = pp.tile([P, MC, S], F16)
            kt_sb = pp.tile([P, MC, S], F16)
            for s in range(NQ):
                for t, (w_sb, dst, boff) in enumerate(
                    [(wq_sb, qt_sb, 0), (wk_sb, kt_sb, 3)]
                ):
                    for m in range(MC):
                        ps = psA.tile(
                            [P, QT], F32, tag="mm", name=f"psp{t}_{m}_{s}"
                        )
                        for c in range(KC):
                            nc.tensor.matmul(
                                ps,
                                lhsT=w_sb[:, c, m * P : (m + 1) * P],
                                rhs=x_sb[:, c, s * QT : (s + 1) * QT],
                                start=(c == 0),
                                stop=(c == KC - 1),
                            )
                        nc.scalar.activation(
                            dst[:, m, s * QT : (s + 1) * QT],
                            ps,
                            IDENT,
                            bias=bqk_sb[:, boff + m : boff + m + 1],
                        )

            # V bias broadcast to all partitions: bvb[p, n] = bv[n]
            bvb_sb = pp.tile([P, DD], F32)
            nc.gpsimd.partition_broadcast(bvb_sb, bvr_sb)

            # ---- attention state ----
            e_store = {}   # j -> E tiles
            cx_store = {}  # j -> (cA, cB) ctx psums
            rd_store = {}  # j -> 1/denom fp16 rows
            ct_store = {}  # j -> normalized fp16 ctx tiles
            mask_q = []    # deferred (e, r) causal-mask multiplies

            def emit_scores_i(j, i, qs):
                """Scores + exp for key tile i of q-tile j.  The causal mask
                multiply is deferred (flush_masks) and runs on GpSimd so it
                stays off both the ACT and DVE critical queues."""
                ks = slice(i * P, (i + 1) * P)
                sps = [
                    psA.tile([P, QT], F32, tag="mm", name=f"sc{h}_{j}_{i}")
                    for h in range(2)
                ]
                # K=128 pieces, then the two K=64 pieces back-to-back
                # (different PE row groups -> they overlap in the array)
                for pi in range(2):
                    for h in range(2):
                        c, p0, pl = HEAD_PIECES[h][pi]
                        nc.tensor.matmul(
                            sps[h],
                            lhsT=kt_sb[p0 : p0 + pl, c, ks],
                            rhs=qt_sb[p0 : p0 + pl, c, qs],
                            start=(pi == 0),
                            stop=(pi == 1),
                        )
                r = i - 4 * j
                pair = []
                for h in range(2):
                    e = ep.tile([P, QT], F16, tag="e", name=f"e{h}_{j}_{i}")
                    nc.scalar.activation(e, sps[h], EXP, scale=SCALE)
                    if r >= 0:
                        mask_q.append((e, r))
                    pair.append(e)
                return pair

            def flush_masks():
                while mask_q:
                    e, r = mask_q.pop(0)
                    nc.vector.tensor_mul(e, e, msk_sb[:, r, :])

            # ---- V projection, seq-major, ones column at col 128 ----
            # v_sb[:, i, h, :] = [V_d0:128 | ones | V_d128:192]; ctx piece A =
            # cols 0:128, piece B = cols 128:193 (denominator row 0 + 64 V).
            # Query tiles are processed in DESCENDING order, so the longest
            # score phase (j=3, 16 key tiles) interleaves 1:1 with this loop,
            # and every later score phase hides behind a longer ctx phase.
            v_sb = pp.tile([P, NK, 2, DH + 1], F16)
            nc.gpsimd.memset(v_sb[:, :, :, 128:129], 1.0)
            ets3 = []
            qs3 = slice(3 * QT, 4 * QT)
            for i in range(NK):
                ps = psA.tile([P, QT], F32, tag="mm", name=f"psv{i}")
                for c in range(KC):
                    nc.tensor.matmul(
                        ps[:, 0:DD],
                        lhsT=x_sb[:, c, i * P : (i + 1) * P],
                        rhs=wv_sb[:, c, :],
                        start=(c == 0),
                        stop=(c == KC - 1),
                    )
                for h in range(2):
                    nc.vector.tensor_add(
                        v_sb[:, i, h, 0:128],
                        ps[:, h * DH : h * DH + 128],
                        bvb_sb[:, h * DH : h * DH + 128],
                    )
                    nc.vector.tensor_add(
                        v_sb[:, i, h, 129 : DH + 1],
                        ps[:, h * DH + 128 : (h + 1) * DH],
                        bvb_sb[:, h * DH + 128 : (h + 1) * DH],
                    )
                ets3.append(emit_scores_i(3, i, qs3))
            e_store[3] = ets3
            flush_masks()

            # ---- attention + output projection, software-pipelined ----
            def alloc_ctx(j):
                cA = [
                    psC.tile([P, QT], F32, tag="cA", name=f"cA{h}_{j}", bufs=2)
                    for h in range(2)
                ]
                cB = [
                    psC.tile([65, QT], F32, tag="cB", name=f"cB{h}_{j}", bufs=3)
                    for h in range(2)
                ]
                cx_store[j] = (cA, cB)
                return cA, cB

            def emit_recip(j):
                """1/denom (DVE) broadcast to all partitions (GpSimd).
                Emitted right after the cB chains stop so the whole chain
                overlaps the rest of the block."""
                _, cB = cx_store[j]
                bsbs = []
                for h in range(2):
                    rd = wp.tile([1, QT], F32, tag="rd", name=f"rd{h}_{j}")
                    nc.vector.reciprocal(rd, cB[h][0:1, :])
                    bsb = wp.tile([P, QT], F32, tag="bsb", name=f"bsb{h}_{j}")
                    nc.gpsimd.partition_broadcast(bsb, rd)
                    bsbs.append(bsb)
                rd_store[j] = bsbs

            def emit_finish(j):
                """Scale ctx by the broadcast 1/denom (DVE)."""
                cA, cB = cx_store.pop(j)
                bsbs = rd_store.pop(j)
                ctxs = []
                for h in range(2):
                    cta = cp.tile([P, QT], F16, tag="ctA", name=f"ctA{h}_{j}")
                    nc.vector.tensor_mul(cta, cA[h], bsbs[h])
                    ctb = cp.tile([65, QT], F16, tag="ctB", name=f"ctB{h}_{j}")
                    nc.vector.tensor_mul(ctb, cB[h][0:65, :], bsbs[h][0:65, :])
                    ctxs.append((cta, ctb))
                ct_store[j] = ctxs

            def emit_outproj_m(j, ms_list):
                """Output projection + store for query tile j, given m chunks."""
                qs = slice(j * QT, (j + 1) * QT)
                (ctA0, ctB0), (ctA1, ctB1) = ct_store[j]
                for m in ms_list:
                    ms = slice(m * P, (m + 1) * P)
                    po = psA.tile([P, QT], F32, tag="mm", name=f"po{m}_{j}")
                    nc.tensor.matmul(po, lhsT=wo_sb[:, 0, ms], rhs=ctA0, start=True, stop=False)
                    nc.tensor.matmul(po, lhsT=wo_sb[0:65, 1, ms], rhs=ctB0, start=False, stop=False)
                    nc.tensor.matmul(po, lhsT=wo_sb[:, 2, ms], rhs=ctA1, start=False, stop=False)
                    nc.tensor.matmul(po, lhsT=wo_sb[0:65, 3, ms], rhs=ctB1, start=False, stop=True)
                    osb = op_.tile([P, QT], F32, tag="osb", name=f"osb{m}_{j}")
                    nc.vector.tensor_copy(osb, po)
                    nc.sync.dma_start(out=out_t[:, m, qs], in_=osb)

            def emit_cb_chain(jc, ec, cB):
                nk_c = 4 * jc + 4
                for i in range(nk_c):
                    for h in range(2):
                        nc.tensor.matmul(
                            cB[h],
                            lhsT=v_sb[:, i, h, 128 : DH + 1],
                            rhs=ec[i][h],
                            start=(i == 0),
                            stop=(i == nk_c - 1),
                        )

            def emit_ca_i(jc, ec, cA, i):
                nk_c = 4 * jc + 4
                for h in range(2):
                    nc.tensor.matmul(
                        cA[h],
                        lhsT=v_sb[:, i, h, 0:128],
                        rhs=ec[i][h],
                        start=(i == 0),
                        stop=(i == nk_c - 1),
                    )

            def emit_block(js):
                """Score phase of q-tile js with the ctx of q-tile js+1
                interleaved, plus the out-projection of q-tile js+2.  The cB
                (denominator) chains go first so the reciprocal + partition
                broadcast overlap the block; ctx is always 4 key-tiles
                longer than the score phase, so the PE never waits on ACT
                exps, and the finish muls complete during the ctx tail."""
                jc = js + 1
                nk_s = 4 * js + 4
                nk_c = 4 * jc + 4
                qs = slice(js * QT, (js + 1) * QT)
                ec = e_store.pop(jc)
                cA, cB = alloc_ctx(jc)
                emit_cb_chain(jc, ec, cB)
                if jc + 1 in ct_store:
                    emit_outproj_m(jc + 1, list(range(KC)))
                    ct_store.pop(jc + 1)
                emit_recip(jc)
                ets = []
                for i in range(nk_s):
                    emit_ca_i(jc, ec, cA, i)
                    ets.append(emit_scores_i(js, i, qs))
                for i in range(nk_s, nk_c):
                    emit_ca_i(jc, ec, cA, i)
                emit_finish(jc)
                e_store[js] = ets
                flush_masks()

            for js in range(NQ - 2, -1, -1):  # 2, 1, 0
                emit_block(js)
            # epilogue: ctx(0) (4 key tiles) + the two pending projections.
            e0 = e_store.pop(0)
            cA, cB = alloc_ctx(0)
            emit_cb_chain(0, e0, cB)
            emit_outproj_m(1, list(range(KC)))
            ct_store.pop(1)
            emit_recip(0)
            for i in range(4):
                emit_ca_i(0, e0, cA, i)
            emit_finish(0)
            emit_outproj_m(0, list(range(KC)))
            ct_store.pop(0)

    nc.compile()
    return nc


def _get_nc():
    if "nc" not in _CACHE:
        _CACHE["nc"] = _build_nc()
    return _CACHE["nc"]


def _masks():
    kk = np.arange(P)[:, None, None]
    r = np.arange(4)[None, :, None]
    qq = np.arange(QT)[None, None, :]
    return (qq >= kk + P * r).astype(np.float16)


def host_prep(x, Wq, bq, Wk, bk, Wv, bv, Wo):
    """Build the 8 per-core input maps (core c: batch c//2, head-pair c%2)."""
    f16 = np.float16
    x = np.asarray(x, dtype=np.float32)
    Wq, Wk, Wv, Wo = (np.asarray(a, dtype=np.float32) for a in (Wq, Wk, Wv, Wo))
    bq, bk, bv = (np.asarray(a, dtype=np.float32) for a in (bq, bk, bv))
    masks = _masks()
    xt16 = {}
    for b in range(4):
        xt16[b] = np.ascontiguousarray(
            x[b].T.reshape(KC, P, S).transpose(1, 0, 2)
        ).astype(f16)
    in_maps = []
    for c in range(8):
        b, hp = divmod(c, 2)
        cs = slice(hp * DD, (hp + 1) * DD)
        wq16 = np.ascontiguousarray(
            Wq[:, cs].reshape(KC, P, DD).transpose(1, 0, 2)
        ).astype(f16)
        wk16 = np.ascontiguousarray(
            Wk[:, cs].reshape(KC, P, DD).transpose(1, 0, 2)
        ).astype(f16)
        wv16 = np.ascontiguousarray(
            Wv[:, cs].reshape(KC, P, DD).transpose(1, 0, 2)
        ).astype(f16)
        wo_s = Wo[cs, :]
        woc = np.zeros((P, 4, D), np.float32)
        woc[:, 0, :] = wo_s[0:128]
        woc[1:65, 1, :] = wo_s[128:192]
        woc[:, 2, :] = wo_s[192:320]
        woc[1:65, 3, :] = wo_s[320:384]
        bqk_c = np.concatenate(
            [bq[cs].reshape(MC, P).T, bk[cs].reshape(MC, P).T], axis=1
        ).astype(np.float32)
        in_maps.append(
            {
                "xt": xt16[b],
                "wq": wq16,
                "wk": wk16,
                "wv": wv16,
                "wo": woc.astype(f16),
                "bqk": np.ascontiguousarray(bqk_c),
                "bvr": np.ascontiguousarray(bv[cs].reshape(1, DD)).astype(np.float32),
                "msk": masks,
            }
        )
    return in_maps


def combine(per_core_out, bo):
    """Sum the per-batch core pairs and undo the transposed layout."""
    bo = np.asarray(bo, dtype=np.float32)
    out = np.empty((4, S, D), np.float32)
    for b in range(4):
        pt = per_core_out[2 * b] + per_core_out[2 * b + 1]  # [P, KC, S]
        out[b] = pt.transpose(1, 0, 2).reshape(D, S).T + bo
    return out


def run(inp, trace=False):
    from concourse.bass_utils import run_bass_kernel_spmd

    nc = _get_nc()
    in_maps = host_prep(
        inp["inputs"], inp["Wq"], inp["bq"], inp["Wk"], inp["bk"],
        inp["Wv"], inp["bv"], inp["Wo"],
    )
    kw = {}
    if trace:
        kw = dict(trace=True, trace_cores=list(range(8)))
    res = run_bass_kernel_spmd(nc, in_maps, core_ids=list(range(8)), **kw)
    out = combine([r["out_t"] for r in res.results], inp["bo"])
    return out, res


def kernel(inputs, Wq, bq, Wk, bk, Wv, bv, Wo, bo):
    out, _ = run(
        {"inputs": inputs, "Wq": Wq, "bq": bq, "Wk": Wk, "bk": bk,
         "Wv": Wv, "bv": bv, "Wo": Wo, "bo": bo}
    )
    return out



# revision 4
# speedup vs baseline: 1.1935x; 1.1935x over previous
"""Multi-head causal attention (B=4, S=2048, D=768, H=4 heads) on 8 TRN2 cores.

Sharding: core c handles batch b = c//2 and head-pair hp = c%2 (heads 2*hp,
2*hp+1).  Each core projects x[b] through its 384-column slice of Wq/Wk/Wv,
runs causal attention for its two heads, and pushes the result through its
384-row slice of Wo.  The host sums the two partial outputs per batch and
adds bo (+ bv @ Wo: softmax weights sum to 1, so the V bias contributes a
constant row that is folded into the output bias on the host).  This splits
every matmul's FLOPs exactly 8 ways with no duplicated work and needs no
device collectives.

Dataflow is kept transposed end-to-end ([feature, seq] layouts) so the kernel
needs zero on-device transposes:
  QT/KT = W^T x^T           [384, S]   (3 chunks of 128 partitions)
  V     = x W               [S, 384]   (16 chunks of 128 partitions, with a
                                        ones column appended per head so the
                                        softmax denominator falls out of the
                                        ctx matmul as one extra output row)
  S^T   = KT'Q              [k, q]     k on partitions -> softmax sum over k
  ctx^T = V^T E             [192+1, q]
  out^T = Wo^T ctx^T        [768, S]
Causal structure: key-tile i (128 rows) x query-tile j (512 cols) blocks with
i > 4j+3 are skipped entirely; diagonal blocks (r = i-4j in 0..3) only compute
columns 128r.. (the rest are fully masked) and get a 0/1 triangle-mask
multiply on the single partial 128x128 block.  Scores are O(1) so exp needs
no max-subtraction.

The softmax denominator sits in row 64 of the 65-row ctx "B" psum chains
(ones column at V slot 192).  After normalization the two heads' 64-row tails
are repacked (SBUF->SBUF DMA) into one 128-partition tile so the output
projection contracts in exactly 3 K=128 matmuls.

The query-tile loop is software-pipelined: each block runs the cB chains
first (denominator -> fast-approx reciprocal -> partition broadcast overlap
the block), gives the cA chains a 4-tile head start so the normalization
muls overlap the score tail, and interleaves the previous tile's output
projection + fp16 stores.  Matmul operands are fp16 (PSUM accumulates fp32).
"""

import sys

for _p in ("/opt/trn_rl_repo",):
    if _p not in sys.path:
        sys.path.insert(0, _p)

import numpy as np

S = 2048            # sequence length
D = 768             # model dim
DH = 192            # head dim
DD = 2 * DH         # feature columns per core (2 heads)
P = 128             # partitions
KC = D // P         # 6 contraction chunks over D
MC = DD // P        # 3 chunks over the per-core head dims
QT = 512            # query tile (matmul free dim, one PSUM bank)
NQ = S // QT        # 4 query tiles
NK = S // P         # 16 key tiles
SCALE = 1.0 / float(np.sqrt(DH))
NWARM = 32          # PE p-state warmup matmuls during the load phase

# Per-head slices of the [384 -> 3x128chunk] QT/KT layout, ordered so the two
# K=64 pieces of the two heads land in different PE row groups.
#   h=0: chunk0 rows 0:128  +  chunk1 rows 0:64
#   h=1: chunk2 rows 0:128  +  chunk1 rows 64:128
HEAD_PIECES = [
    [(0, 0, 128), (1, 0, 64)],
    [(2, 0, 128), (1, 64, 64)],
]

_CACHE = {}


def _build_nc():
    import concourse.bacc as bacc
    import concourse.tile as tile
    from concourse import mybir

    F16 = mybir.dt.float16
    F32 = mybir.dt.float32
    EXP = mybir.ActivationFunctionType.Exp
    IDENT = mybir.ActivationFunctionType.Identity
    COPY = mybir.ActivationFunctionType.Copy

    nc = bacc.Bacc(None, target_bir_lowering=False)

    xt = nc.dram_tensor("xt", [P, KC, S], F16, kind="ExternalInput")
    wq = nc.dram_tensor("wq", [P, MC, D], F16, kind="ExternalInput")
    wk = nc.dram_tensor("wk", [P, MC, D], F16, kind="ExternalInput")
    wv = nc.dram_tensor("wv", [P, KC, DD], F16, kind="ExternalInput")
    wo = nc.dram_tensor("wo", [P, 3, D], F16, kind="ExternalInput")
    bqk = nc.dram_tensor("bqk", [P, 6], F32, kind="ExternalInput")
    msk = nc.dram_tensor("msk", [P, P], F16, kind="ExternalInput")
    out_t = nc.dram_tensor("out_t", [P, KC, S], F16, kind="ExternalOutput")

    with tile.TileContext(nc) as tc:
        with (
            tc.tile_pool(name="persist", bufs=1) as pp,
            tc.tile_pool(name="epool", bufs=56) as ep,
            tc.tile_pool(name="ctxp", bufs=4) as cp,
            tc.tile_pool(name="workp", bufs=2) as wp,
            tc.tile_pool(name="outp", bufs=3) as op_,
            tc.tile_pool(name="psA", bufs=3, space="PSUM") as psA,
            tc.tile_pool(name="psC", bufs=2, space="PSUM") as psC,
        ):
            # ---- loads, split across both HWDGE rings (sync + scalar).
            # The first QK-proj matmuls need wq chunk 0 + the first x quarter;
            # those go first (wq is m-chunked so chunk 0 lands soonest).
            x_sb = pp.tile([P, KC, S], F16)
            wq_sb = pp.tile([P, MC, D], F16)
            wk_sb = pp.tile([P, MC, D], F16)
            bqk_sb = pp.tile([P, 6], F32)
            for m in range(MC):
                nc.sync.dma_start(out=wq_sb[:, m, :], in_=wq[:, m, :])
            nc.sync.dma_start(out=x_sb[:, 0:3, 0:QT], in_=xt[:, 0:3, 0:QT])
            nc.scalar.dma_start(out=x_sb[:, 3:6, 0:QT], in_=xt[:, 3:6, 0:QT])
            nc.scalar.dma_start(out=bqk_sb, in_=bqk[:, :])
            nc.scalar.dma_start(out=wk_sb, in_=wk[:, :, :])
            nc.sync.dma_start(out=x_sb[:, :, QT : 2 * QT], in_=xt[:, :, QT : 2 * QT])
            nc.scalar.dma_start(
                out=x_sb[:, :, 2 * QT : 3 * QT], in_=xt[:, :, 2 * QT : 3 * QT]
            )
            wv_sb = pp.tile([P, KC, DD], F16)
            nc.scalar.dma_start(out=wv_sb, in_=wv[:, :, :])
            nc.sync.dma_start(out=x_sb[:, :, 3 * QT : S], in_=xt[:, :, 3 * QT : S])
            wo_sb = pp.tile([P, 3, D], F16)
            nc.scalar.dma_start(out=wo_sb, in_=wo[:, :, :])
            msk_sb = pp.tile([P, P], F16)
            nc.scalar.dma_start(out=msk_sb, in_=msk[:, :])

            ones_sb = pp.tile([1, P], F16)
            nc.vector.memset(ones_sb, 1.0)

            # Preload the ACT exp table during the load phase so the first
            # real exp doesn't pay the table-load latency.
            warm_act = pp.tile([1, 16], F32)
            nc.scalar.activation(warm_act, ones_sb[:, 0:16], EXP, scale=1.0)

            # Dummy matmuls on the just-memset ones tile: keeps TensorE busy
            # during the initial loads so the p-state ramp is already done
            # when the first real matmul issues.
            ps_warm = psA.tile([P, QT], F32, tag="mm", name="ps_warm")
            for w in range(NWARM):
                nc.tensor.matmul(
                    ps_warm[:, 0:P], lhsT=ones_sb, rhs=ones_sb,
                    start=True, stop=True, skip_group_check=(w > 0),
                )

            # ---- Q^T and K^T projections: [384(3x128), 2048] fp16 ----
            # (bias-add + fp16 cast on ScalarE to keep DVE free)
            qt_sb = pp.tile([P, MC, S], F16)
            kt_sb = pp.tile([P, MC, S], F16)
            for s in range(NQ):
                for t, (w_sb, dst, boff) in enumerate(
                    [(wq_sb, qt_sb, 0), (wk_sb, kt_sb, 3)]
                ):
                    for m in range(MC):
                        ps = psA.tile(
                            [P, QT], F32, tag="mm", name=f"psp{t}_{m}_{s}"
                        )
                        for c in range(KC):
                            nc.tensor.matmul(
                                ps,
                                lhsT=w_sb[:, m, c * P : (c + 1) * P],
                                rhs=x_sb[:, c, s * QT : (s + 1) * QT],
                                start=(c == 0),
                                stop=(c == KC - 1),
                            )
                        nc.scalar.activation(
                            dst[:, m, s * QT : (s + 1) * QT],
                            ps,
                            IDENT,
                            bias=bqk_sb[:, boff + m : boff + m + 1],
                        )

            # ---- attention state ----
            e_store = {}   # j -> E tiles
            cx_store = {}  # j -> (cA, cB) ctx psums
            rd_store = {}  # j -> broadcast 1/denom tiles
            ct_store = {}  # j -> (ctA0, ctP, ctA1) normalized fp16 ctx tiles
            mask_q = []    # deferred (e, r) causal-mask multiplies

            def emit_scores_i(j, i):
                """Scores + exp for key tile i of q-tile j.  Diagonal tiles
                (r = i-4j >= 0) only compute columns >= 128r; the partial
                128x128 block gets a deferred triangle-mask multiply."""
                r = i - 4 * j
                off = P * r if r > 0 else 0
                ks = slice(i * P, (i + 1) * P)
                qs = slice(j * QT + off, (j + 1) * QT)
                sps = [
                    psA.tile([P, QT], F32, tag="mm", name=f"sc{h}_{j}_{i}")
                    for h in range(2)
                ]
                # K=128 pieces, then the two K=64 pieces
                for pi in range(2):
                    for h in range(2):
                        c, p0, pl = HEAD_PIECES[h][pi]
                        nc.tensor.matmul(
                            sps[h][:, off:],
                            lhsT=kt_sb[p0 : p0 + pl, c, ks],
                            rhs=qt_sb[p0 : p0 + pl, c, qs],
                            start=(pi == 0),
                            stop=(pi == 1),
                        )
                pair = []
                for h in range(2):
                    e = ep.tile([P, QT], F16, tag="e", name=f"e{h}_{j}_{i}")
                    nc.scalar.activation(e[:, off:], sps[h][:, off:], EXP, scale=SCALE)
                    if r >= 0:
                        mask_q.append((e, r))
                    pair.append(e)
                return pair

            def flush_masks():
                while mask_q:
                    e, r = mask_q.pop(0)
                    off = P * r
                    nc.vector.tensor_mul(
                        e[:, off : off + P], e[:, off : off + P], msk_sb
                    )

            def coff(j, i):
                r = i - 4 * j
                return P * r if r > 0 else 0

            # ---- V projection, seq-major, ones column at col 192 ----
            # v_sb[:, i, h, :] = [V_d0:192 | ones]; ctx piece A = cols 0:128,
            # piece B = cols 128:193 (64 V rows + denominator row 64).
            # Query tiles are processed in DESCENDING order, so the longest
            # score phase (j=3, 16 key tiles) interleaves 1:1 with this loop,
            # and every later score phase hides behind a longer ctx phase.
            v_sb = pp.tile([P, NK, 2, DH + 1], F16)
            nc.gpsimd.memset(v_sb[:, :, :, DH : DH + 1], 1.0)
            ets3 = []
            for i in range(NK):
                ps = psA.tile([P, QT], F32, tag="mm", name=f"psv{i}")
                for c in range(KC):
                    nc.tensor.matmul(
                        ps[:, 0:DD],
                        lhsT=x_sb[:, c, i * P : (i + 1) * P],
                        rhs=wv_sb[:, c, :],
                        start=(c == 0),
                        stop=(c == KC - 1),
                    )
                for h in range(2):
                    nc.vector.tensor_copy(
                        v_sb[:, i, h, 0:DH], ps[:, h * DH : (h + 1) * DH]
                    )
                ets3.append(emit_scores_i(3, i))
            e_store[3] = ets3
            flush_masks()

            # ---- attention + output projection, software-pipelined ----
            def alloc_ctx(j):
                cA = [
                    psC.tile([P, QT], F32, tag="cA", name=f"cA{h}_{j}", bufs=2)
                    for h in range(2)
                ]
                cB = [
                    psC.tile([65, QT], F32, tag="cB", name=f"cB{h}_{j}", bufs=3)
                    for h in range(2)
                ]
                cx_store[j] = (cA, cB)
                return cA, cB

            def emit_recip(j):
                """1/denom (fast-approx DVE) broadcast to all partitions
                (GpSimd).  Emitted right after the cB chains stop so the
                whole chain overlaps the rest of the block."""
                _, cB = cx_store[j]
                bsbs = []
                for h in range(2):
                    # ACT copy to SBUF first: custom DVE ops misread PSUM.
                    den = wp.tile([1, QT], F32, tag="den", name=f"den{h}_{j}")
                    nc.scalar.activation(den, cB[h][64:65, :], COPY)
                    rd = wp.tile([1, QT], F32, tag="rd", name=f"rd{h}_{j}")
                    nc.vector.reciprocal_approx_fast(rd, den)
                    bsb = wp.tile([P, QT], F32, tag="bsb", name=f"bsb{h}_{j}")
                    nc.gpsimd.partition_broadcast(bsb, rd)
                    bsbs.append(bsb)
                rd_store[j] = bsbs

            def emit_finish(j):
                """Scale ctx by the broadcast 1/denom (DVE).  The B tails go
                first so their SBUF->SBUF repack DMA (into the packed 128-row
                ctP tile) starts early."""
                cA, cB = cx_store.pop(j)
                bsbs = rd_store.pop(j)
                ctp = cp.tile([P, QT], F16, tag="ctP", name=f"ctP{j}", bufs=3)
                for h in range(2):
                    ctb = cp.tile(
                        [64, QT], F16, tag="ctB", name=f"ctB{h}_{j}", bufs=3
                    )
                    nc.vector.tensor_mul(ctb, cB[h][0:64, :], bsbs[h][0:64, :])
                    nc.gpsimd.dma_start(
                        out=ctp[64 * h : 64 * h + 64, :], in_=ctb
                    )
                ctas = []
                for h in range(2):
                    cta = cp.tile(
                        [P, QT], F16, tag="ctA", name=f"ctA{h}_{j}", bufs=4
                    )
                    nc.vector.tensor_mul(cta, cA[h], bsbs[h])
                    ctas.append(cta)
                ct_store[j] = (ctas[0], ctp, ctas[1])

            def emit_outproj_m(j, ms_list):
                """Output projection + fp16 store for query tile j."""
                qs = slice(j * QT, (j + 1) * QT)
                ctA0, ctP, ctA1 = ct_store[j]
                for m in ms_list:
                    ms = slice(m * P, (m + 1) * P)
                    po = psA.tile([P, QT], F32, tag="mm", name=f"po{m}_{j}")
                    nc.tensor.matmul(po, lhsT=wo_sb[:, 1, ms], rhs=ctP, start=True, stop=False)
                    nc.tensor.matmul(po, lhsT=wo_sb[:, 0, ms], rhs=ctA0, start=False, stop=False)
                    nc.tensor.matmul(po, lhsT=wo_sb[:, 2, ms], rhs=ctA1, start=False, stop=True)
                    osb = op_.tile([P, QT], F16, tag="osb", name=f"osb{m}_{j}")
                    nc.vector.tensor_copy(osb, po)
                    eng = nc.sync if (m % 2 == 0) else nc.scalar
                    eng.dma_start(out=out_t[:, m, qs], in_=osb)

            def emit_cb_chain(jc, ec, cB):
                nk_c = 4 * jc + 4
                for i in range(nk_c):
                    off = coff(jc, i)
                    for h in range(2):
                        nc.tensor.matmul(
                            cB[h][:, off:],
                            lhsT=v_sb[:, i, h, 128 : DH + 1],
                            rhs=ec[i][h][:, off:],
                            start=(i == 0),
                            stop=(i == nk_c - 1),
                        )

            def emit_ca_i(jc, ec, cA, i):
                nk_c = 4 * jc + 4
                off = coff(jc, i)
                for h in range(2):
                    nc.tensor.matmul(
                        cA[h][:, off:],
                        lhsT=v_sb[:, i, h, 0:128],
                        rhs=ec[i][h][:, off:],
                        start=(i == 0),
                        stop=(i == nk_c - 1),
                    )

            def emit_block(js):
                """Score phase of q-tile js with the ctx of q-tile js+1
                interleaved, plus the out-projection of q-tile js+2.  The cB
                (denominator) chains go first so the reciprocal + partition
                broadcast overlap the block; the cA chains get a 4-key-tile
                head start so they finish before the score tail and the
                normalization muls overlap the last scores."""
                jc = js + 1
                nk_s = 4 * js + 4
                ec = e_store.pop(jc)
                cA, cB = alloc_ctx(jc)
                emit_cb_chain(jc, ec, cB)
                emit_recip(jc)
                for i in range(4):
                    emit_ca_i(jc, ec, cA, i)
                if jc + 1 in ct_store:
                    emit_outproj_m(jc + 1, list(range(KC)))
                    ct_store.pop(jc + 1)
                ets = []
                for i in range(nk_s):
                    emit_ca_i(jc, ec, cA, i + 4)
                    ets.append(emit_scores_i(js, i))
                emit_finish(jc)
                e_store[js] = ets
                flush_masks()

            for js in range(NQ - 2, -1, -1):  # 2, 1, 0
                emit_block(js)
            # epilogue: ctx(0) (4 key tiles) + the two pending projections.
            e0 = e_store.pop(0)
            cA, cB = alloc_ctx(0)
            emit_cb_chain(0, e0, cB)
            emit_recip(0)
            for i in range(4):
                emit_ca_i(0, e0, cA, i)
            emit_outproj_m(1, list(range(KC)))
            ct_store.pop(1)
            emit_finish(0)
            emit_outproj_m(0, list(range(KC)))
            ct_store.pop(0)

    nc.compile()
    return nc


def _get_nc():
    if "nc" not in _CACHE:
        _CACHE["nc"] = _build_nc()
    return _CACHE["nc"]


def _mask():
    kk = np.arange(P)[:, None]
    qq = np.arange(P)[None, :]
    return (qq >= kk).astype(np.float16)


def host_prep(x, Wq, bq, Wk, bk, Wv, Wo):
    """Build the 8 per-core input maps (core c: batch c//2, head-pair c%2)."""
    f16 = np.float16
    x = np.asarray(x, dtype=np.float32)
    Wq, Wk, Wv, Wo = (np.asarray(a, dtype=np.float32) for a in (Wq, Wk, Wv, Wo))
    bq, bk = (np.asarray(a, dtype=np.float32) for a in (bq, bk))
    mask = _mask()
    xt16 = {}
    for b in range(4):
        xt16[b] = np.ascontiguousarray(
            x[b].T.reshape(KC, P, S).transpose(1, 0, 2)
        ).astype(f16)

    def pack_m(W, cs):
        # [p, m, c*128+col] = W[c*128+p, cs.start + m*128+col]
        return np.ascontiguousarray(
            W[:, cs].reshape(KC, P, MC, P).transpose(1, 2, 0, 3).reshape(P, MC, D)
        ).astype(f16)

    in_maps = []
    for c in range(8):
        b, hp = divmod(c, 2)
        cs = slice(hp * DD, (hp + 1) * DD)
        wv16 = np.ascontiguousarray(
            Wv[:, cs].reshape(KC, P, DD).transpose(1, 0, 2)
        ).astype(f16)
        wo_s = Wo[cs, :]
        woc = np.empty((P, 3, D), np.float32)
        woc[:, 0, :] = wo_s[0:128]
        woc[0:64, 1, :] = wo_s[128:192]
        woc[64:128, 1, :] = wo_s[320:384]
        woc[:, 2, :] = wo_s[192:320]
        bqk_c = np.concatenate(
            [bq[cs].reshape(MC, P).T, bk[cs].reshape(MC, P).T], axis=1
        ).astype(np.float32)
        in_maps.append(
            {
                "xt": xt16[b],
                "wq": pack_m(Wq, cs),
                "wk": pack_m(Wk, cs),
                "wv": wv16,
                "wo": woc.astype(f16),
                "bqk": np.ascontiguousarray(bqk_c),
                "msk": mask,
            }
        )
    return in_maps


def combine(per_core_out, bo_eff):
    """Sum the per-batch core pairs and undo the transposed layout."""
    out = np.empty((4, S, D), np.float32)
    for b in range(4):
        pt = per_core_out[2 * b].astype(np.float32) + per_core_out[
            2 * b + 1
        ].astype(np.float32)  # [P, KC, S]
        out[b] = pt.transpose(1, 0, 2).reshape(D, S).T + bo_eff
    return out


def run(inp, trace=False):
    from concourse.bass_utils import run_bass_kernel_spmd

    nc = _get_nc()
    in_maps = host_prep(
        inp["inputs"], inp["Wq"], inp["bq"], inp["Wk"], inp["bk"],
        inp["Wv"], inp["Wo"],
    )
    # V bias: softmax weights sum to 1, so ctx = ctx_nobias + bv per head;
    # the whole bv contribution collapses into an output-bias correction.
    bo_eff = (
        np.asarray(inp["bo"], np.float64)
        + np.asarray(inp["bv"], np.float64) @ np.asarray(inp["Wo"], np.float64)
    ).astype(np.float32)
    kw = {}
    if trace:
        kw = dict(trace=True, trace_cores=list(range(8)))
    res = run_bass_kernel_spmd(nc, in_maps, core_ids=list(range(8)), **kw)
    out = combine([r["out_t"] for r in res.results], bo_eff)
    return out, res


def kernel(inputs, Wq, bq, Wk, bk, Wv, bv, Wo, bo):
    out, _ = run(
        {"inputs": inputs, "Wq": Wq, "bq": bq, "Wk": Wk, "bk": bk,
         "Wv": Wv, "bv": bv, "Wo": Wo, "bo": bo}
    )
    return out
